# revision 43
# baseline (speedup 1.0000x reference)
"""Trainium2 Bass kernel for the autoregressive LSTM decoder.

Problem: B=64, T=512 decoder steps, latent L=256, hidden H=1024.
tf_prob=0 and the per-step uniform draws (key 42) are all > 0, so the
decoder is purely autoregressive: targets is never used and the input
matmul folds into the hidden matmul:

    x_{t+1} = out_t = h_t @ w_fc.T + b_fc
    gates_{t+1} = x_{t+1} @ w_ih.T + h_t @ w_hh.T + b
                = h_t @ (w_fc.T @ w_ih.T + w_hh.T) + (b + b_fc @ w_ih.T)
                = h_t @ W_eff + b_eff

Step 0 (which uses initial_input / h0 / c0) is computed on the host in
fp32; the device runs steps 1..511 of the collapsed recurrence and the
fc output projection.

Device layout (batch-major gates):
  - h kept transposed (h_T: H on partitions) as 8 per-chunk tiles
    [128, 64] so it can be the stationary matmul operand, and so the
    next step's k-th matmul only depends on chunk k.
  - W_eff columns permuted per h-chunk: [i_k | f_k | o_k | g_k] (128
    cols each) so chunk k's 512 gate columns are contiguous and the
    sigmoid slab [64, 384] / tanh slab [64, 128] are contiguous.
  - gates chunk = PSUM [64, 512] accumulated over 8 K-tiles + a K=1
    "ones row" matmul that adds the bias.
  - bf16 (default): chunk pairs run in the two PE column groups
    concurrently (tile_position col tiling, separate PSUM banks; odd
    chunks live at PSUM/compute partitions 64..127) -> ~2x matmul
    throughput. fp32-class dtypes cannot be column-tiled (f32r build
    keeps one chunk at a time).
  - nonlinearities on ACT (sigmoid/tanh), c/h updates on DVE, h chunk
    transposed back to h_T via PE transpose (identity has an I64 block
    in each partition half).

The recurrence is strictly sequential and PE-bound on one core; a
per-step cross-core exchange (ncfw collective floor ~5us) costs more
than the ~1-2us/step it could save, so all 8 cores run the same program
(data-parallel-degenerate) and core 0's output is used.
"""

import os
import numpy as np

B, T, L, H = 64, 512, 256, 1024
P = 128
NK = H // P            # 8 k-tiles
G4 = 4 * H             # 4096 gate cols
NCH = NK               # 8 gate chunks of 512 cols

_prog_cache = {}


def _gate_perm():
    """Column permutation of [4H] gate space -> per-chunk [i|f|o|g]."""
    perm = np.empty(G4, np.int64)
    pos = 0
    for k in range(NK):
        for gidx in (0, 1, 3, 2):   # i, f, o, g  (torch order i,f,g,o)
            base = gidx * H + k * P
            perm[pos:pos + P] = np.arange(base, base + P)
            pos += P
    return perm


def _build_program(mm_dt_name: str, n_steps: int = T):
    import concourse.bass as bass
    import concourse.bacc as bacc
    import concourse.mybir as mybir
    from concourse.bass import ds, ts
    from concourse.tile import TileContext
    from concourse.masks import make_identity

    f32 = mybir.dt.float32
    if mm_dt_name == "bf16":
        mm_store_dt = mybir.dt.bfloat16
    elif mm_dt_name == "f32r":
        mm_store_dt = mybir.dt.float32r
    else:
        mm_store_dt = f32
    # PE column tiling (two concurrent chunk matmuls) is only legal for
    # non-fp32-class dtypes.
    tile2 = mm_dt_name == "bf16"
    tr_dt0 = mm_store_dt if tile2 else f32  # transpose/hbm dtype

    def mm(ap):
        return ap

    nc = bacc.Bacc(None, target_bir_lowering=False)
    AF = mybir.ActivationFunctionType

    # ---- DRAM I/O ----
    hT0 = nc.declare_dram_parameter("hT0", [H, B], mm_store_dt, isOutput=False)
    c0_shape = [P, H // 2] if tile2 else [B, H]
    c0 = nc.declare_dram_parameter("c0", c0_shape, f32, isOutput=False)
    Wg = nc.declare_dram_parameter("Wg", [H, G4], mm_store_dt, isOutput=False)
    bg = nc.declare_dram_parameter("bg", [1, G4], mm_store_dt, isOutput=False)
    Wf = nc.declare_dram_parameter("Wf", [H, L], mm_store_dt, isOutput=False)
    bf = nc.declare_dram_parameter("bf", [1, L], mm_store_dt, isOutput=False)
    ones_d = nc.declare_dram_parameter("ones", [1, B], mm_store_dt, isOutput=False)
    ident_d = nc.declare_dram_parameter("ident2", [P, B], tr_dt0, isOutput=False)
    # outs[t] for t=1..511 stored as [(t-1)*64 .. t*64) rows; one extra
    # (discarded) row-block so the loop can run a full 256 pairs of steps.
    outs = nc.declare_dram_parameter("outs", [T * B, L], f32, isOutput=True)

    with TileContext(nc) as tc:
        with (
            tc.tile_pool(name="consts", bufs=1) as consts,
            tc.tile_pool(name="state", bufs=1) as state,
            tc.tile_pool(name="work", bufs=int(os.environ.get("BASS_LSTM_WORKBUFS", "3"))) as work,
            tc.tile_pool(name="psumG", bufs=2, space="PSUM") as psumG,
            tc.tile_pool(name="psumS", bufs=2, space="PSUM") as psumS,
        ):
            # ---- constants / weights resident in SBUF ----
            W_sb = consts.tile([P, NK * G4], mm_store_dt, tag="W")
            for k in range(NK):
                nc.sync.dma_start(
                    out=W_sb[:, k * G4:(k + 1) * G4],
                    in_=Wg[k * P:(k + 1) * P, :],
                )
            Wf_sb = consts.tile([P, NK * L], mm_store_dt, tag="Wf")
            for k in range(NK):
                nc.sync.dma_start(
                    out=Wf_sb[:, k * L:(k + 1) * L],
                    in_=Wf[k * P:(k + 1) * P, :],
                )
            bg_sb = consts.tile([1, G4], mm_store_dt, tag="bg")
            nc.sync.dma_start(out=bg_sb[:], in_=bg[:])
            bf_sb = consts.tile([1, L], mm_store_dt, tag="bf")
            nc.sync.dma_start(out=bf_sb[:], in_=bf[:])
            ones_row = consts.tile([1, B], mm_store_dt, tag="ones")
            nc.sync.dma_start(out=ones_row[:], in_=ones_d[:])
            # identity with an I64 block in each partition half (for
            # transposes of inputs living at partition offset 0 or 64);
            # dtype matches the transpose input (hbm)
            tr_dt = tr_dt0
            ident2 = consts.tile([P, B], tr_dt, tag="ident")
            nc.sync.dma_start(out=ident2[:], in_=ident_d[:])

            # h kept as 8 separate per-chunk tiles so cross-step dependencies
            # are per-chunk (next step's k-th matmul only waits on chunk k)
            hA = [state.tile([P, B], mm_store_dt, tag=f"hA{k}", name=f"hA{k}") for k in range(NK)]
            hB = [state.tile([P, B], mm_store_dt, tag=f"hB{k}", name=f"hB{k}") for k in range(NK)]
            # c split into 4 column-block tiles (tile2: [128,128] holding an
            # even chunk in partitions 0..63 and an odd chunk in 64..127;
            # plain: 8 tiles [64,128])
            if tile2:
                c_tiles = [state.tile([P, P], f32, tag=f"c{q}", name=f"c{q}") for q in range(4)]
            else:
                c_tiles = [state.tile([B, P], f32, tag=f"c{q}", name=f"c{q}") for q in range(NK)]
            for k in range(NK):
                nc.sync.dma_start(out=hA[k][:], in_=hT0[k * P:(k + 1) * P, :])
            for q, ct in enumerate(c_tiles):
                nc.sync.dma_start(out=ct[:], in_=c0[:, q * P:(q + 1) * P])

            def nonlin(G, ps, ct, hdst_k):
                """sigmoid/tanh + c/h update for one chunk.

                ps: partition slice (0:64 or 64:128); ct: this chunk's c tile;
                hdst_k: destination h_T tile [128, 64]."""
                sig = work.tile([P, 384], f32, tag="sig")
                nc.scalar.activation(sig[ps, :], G[ps, 0:384], AF.Sigmoid)
                tg = work.tile([P, P], f32, tag="tg")
                nc.scalar.activation(tg[ps, :], G[ps, 384:512], AF.Tanh)
                t1 = work.tile([P, P], f32, tag="t1")
                nc.vector.tensor_mul(t1[ps, :], sig[ps, 0:P], tg[ps, :])
                t2 = work.tile([P, P], f32, tag="t2")
                nc.vector.tensor_mul(t2[ps, :], sig[ps, P:2 * P], ct[ps, :])
                nc.vector.tensor_add(ct[ps, :], t1[ps, :], t2[ps, :])
                tc2 = work.tile([P, P], f32, tag="tc2")
                nc.scalar.activation(tc2[ps, :], ct[ps, :], AF.Tanh)
                hbm = work.tile([P, P], tr_dt, tag="hbm")
                nc.vector.tensor_mul(hbm[ps, :], sig[ps, 2 * P:3 * P], tc2[ps, :])
                pt = psumS.tile([P, B], tr_dt, tag="pt")
                nc.tensor.transpose(pt[:], hbm[ps, :], ident2[ps, :])
                nc.vector.tensor_copy(hdst_k[:], pt[:])

            def step_tiled(hsrc, hdst):
                """Chunk pairs: even chunk in PE column group 0 (PSUM
                partitions 0..63), odd chunk in group 1 (64..127); the two
                groups' matmuls stream concurrently via different XBUSes."""
                for cq in range(NCH // 2):
                    che, cho = 2 * cq, 2 * cq + 1
                    # separate PSUM banks per column group: a start=True
                    # clears bank-wide, so the groups cannot share a bank
                    G0 = psumG.tile([P, 512], f32, tag="G0")
                    G1 = psumG.tile([P, 512], f32, tag="G1")
                    for k in range(NK):
                        nc.tensor.matmul(
                            G0[0:B, :],
                            lhsT=mm(hsrc[k][:]),
                            rhs=mm(W_sb[:, k * G4 + che * 512: k * G4 + (che + 1) * 512]),
                            start=(k == 0), stop=False, tile_position=(0, 0),
                        )
                        nc.tensor.matmul(
                            G1[B:P, :],
                            lhsT=mm(hsrc[k][:]),
                            rhs=mm(W_sb[:, k * G4 + cho * 512: k * G4 + (cho + 1) * 512]),
                            start=(k == 0), stop=False, tile_position=(0, B),
                        )
                    nc.tensor.matmul(
                        G0[0:B, :], lhsT=mm(ones_row[:]),
                        rhs=mm(bg_sb[:, che * 512:(che + 1) * 512]),
                        start=False, stop=True, tile_position=(0, 0),
                    )
                    nc.tensor.matmul(
                        G1[B:P, :], lhsT=mm(ones_row[:]),
                        rhs=mm(bg_sb[:, cho * 512:(cho + 1) * 512]),
                        start=False, stop=True, tile_position=(0, B),
                    )
                    nonlin(G0, slice(0, B), c_tiles[cq], hdst[che])
                    nonlin(G1, slice(B, P), c_tiles[cq], hdst[cho])

            def step_plain(hsrc, hdst):
                """One chunk at a time, no PE column tiling (fp32/f32r)."""
                for ch in range(NCH):
                    G = psumG.tile([P, 512], f32, tag="G")
                    for k in range(NK):
                        nc.tensor.matmul(
                            G[0:B, :],
                            lhsT=mm(hsrc[k][:]),
                            rhs=mm(W_sb[:, k * G4 + ch * 512: k * G4 + (ch + 1) * 512]),
                            start=(k == 0), stop=False,
                        )
                    nc.tensor.matmul(
                        G[0:B, :], lhsT=mm(ones_row[:]),
                        rhs=mm(bg_sb[:, ch * 512:(ch + 1) * 512]),
                        start=False, stop=True,
                    )
                    nonlin(G, slice(0, B), c_tiles[ch], hdst[ch])

            def step(hsrc, hdst, t_expr):
                if tile2:
                    step_tiled(hsrc, hdst)
                else:
                    step_plain(hsrc, hdst)
                # fc projection out_t = h_t @ w_fc.T + b_fc
                O = psumS.tile([B, L], f32, tag="O")
                for k in range(NK):
                    nc.tensor.matmul(
                        O[:],
                        lhsT=mm(hdst[k][:]),
                        rhs=mm(Wf_sb[:, k * L:(k + 1) * L]),
                        start=(k == 0),
                        stop=False,
                    )
                nc.tensor.matmul(
                    O[:], lhsT=mm(ones_row[:]), rhs=mm(bf_sb[:]), start=False, stop=True
                )
                osb = work.tile([B, L], f32, tag="osb")
                nc.vector.tensor_copy(osb[:], O[:])
                nc.scalar.dma_start(out=outs[ts(t_expr, B), :], in_=osb[:])

            n_pairs = n_steps // 2  # 256 pairs -> steps 1..512 (512 discarded)
            with tc.For_i(0, n_pairs, staggered_reset=True) as i:
                step(hA, hB, i * 2 + 0)   # step t=2i+1 -> outs row block 2i
                step(hB, hA, i * 2 + 1)   # step t=2i+2

    if not nc.is_finalized():
        nc.finalize()
    return nc


def _prepare_host_inputs(initial_input, h0, c0, w_ih, w_hh, b_ih, b_hh, w_fc, b_fc,
                         mm_dt_name):
    """Host: fp32 step 0 + collapsed weights, permuted for the device."""
    import ml_dtypes

    f64 = np.float64
    w_ih64, w_hh64 = w_ih.astype(f64), w_hh.astype(f64)
    w_fc64, b_fc64 = w_fc.astype(f64), b_fc.astype(f64)
    bias64 = b_ih.astype(f64) + b_hh.astype(f64)

    W_eff = (w_fc64.T @ w_ih64.T + w_hh64.T).astype(np.float32)   # [H, 4H]
    b_eff = (bias64 + b_fc64 @ w_ih64.T).astype(np.float32)       # [4H]

    # step 0 in fp32 (matches reference numerics closely)
    def sigmoid(x):
        return 1.0 / (1.0 + np.exp(-x))

    x = initial_input.astype(np.float32)
    h = h0[0].astype(np.float32)
    c = c0[0].astype(np.float32)
    g = x @ w_ih.T.astype(np.float32) + h @ w_hh.T.astype(np.float32) \
        + (bias64.astype(np.float32))
    i_, f_, g_, o_ = np.split(g, 4, axis=1)
    c = sigmoid(f_) * c + sigmoid(i_) * np.tanh(g_)
    h = sigmoid(o_) * np.tanh(c)
    out0 = h @ w_fc.T.astype(np.float32) + b_fc.astype(np.float32)

    perm = _gate_perm()
    Wg = np.ascontiguousarray(W_eff[:, perm])
    bg = np.ascontiguousarray(b_eff[perm])[None, :]
    Wf = np.ascontiguousarray(w_fc.T.astype(np.float32))
    bf = b_fc.astype(np.float32)[None, :]
    hT = np.ascontiguousarray(h.T)

    if mm_dt_name == "bf16":
        cast = lambda a: a.astype(ml_dtypes.bfloat16)
    else:
        cast = lambda a: a.astype(np.float32)

    if mm_dt_name == "bf16":
        # pack c [64, 1024] -> [128, 512]: chunk k at partitions (k%2)*64,
        # cols (k//2)*128
        c_packed = np.zeros((128, H // 2), np.float32)
        for k in range(8):
            c_packed[(k % 2) * B:(k % 2) * B + B,
                     (k // 2) * 128:(k // 2) * 128 + 128] = c[:, k * 128:(k + 1) * 128]
    else:
        c_packed = c.astype(np.float32)

    ident2 = np.zeros((128, B), np.float32)
    ident2[0:B, :] = np.eye(B, dtype=np.float32)
    ident2[B:128, :] = np.eye(B, dtype=np.float32)
    if mm_dt_name == "bf16":
        ident2 = ident2.astype(ml_dtypes.bfloat16)

    in_map = {
        "hT0": cast(hT),
        "c0": np.ascontiguousarray(c_packed),
        "Wg": cast(Wg),
        "bg": cast(bg),
        "Wf": cast(Wf),
        "bf": cast(bf),
        "ones": cast(np.ones((1, B), np.float32)),
        "ident2": ident2,
    }
    return in_map, out0


LAST_EXEC_NS = None

# min over jax.random.uniform(jax.random.key(42), (512,)) — the per-step
# teacher-forcing draws inside the reference. tf_prob below this means the
# decoder is purely autoregressive (the case the device kernel implements).
_RAND_MIN = 5.8138370513916016e-04


def _kernel_numpy_fallback(initial_input, h0, c0, targets, tf_prob,
                           w_ih, w_hh, b_ih, b_hh, w_fc, b_fc):
    """Host fp32 implementation incl. teacher forcing (only used if
    tf_prob >= min(rand), which the problem spec never produces)."""
    import jax
    import jax.numpy as jnp
    rand = np.asarray(jax.random.uniform(jax.random.key(42), (T,), jnp.float32))

    def sigmoid(x):
        return 1.0 / (1.0 + np.exp(-x))

    bias = (b_ih + b_hh).astype(np.float32)
    h = h0[0].astype(np.float32)
    c = c0[0].astype(np.float32)
    inp = initial_input.astype(np.float32)
    outs = []
    for t in range(T):
        g = inp @ w_ih.T + h @ w_hh.T + bias
        i, f, gg, o = np.split(g, 4, axis=1)
        c = sigmoid(f) * c + sigmoid(i) * np.tanh(gg)
        h = sigmoid(o) * np.tanh(c)
        out = h @ w_fc.T + b_fc
        inp = out if rand[t] > tf_prob else targets[:, t, :]
        outs.append(out)
    return np.stack(outs, axis=1)[:, None, :, :].astype(np.float32)


def kernel(initial_input, encoder_outputs, h0, c0, targets, tf_prob,
           w_ih, w_hh, b_ih, b_hh, w_fc, b_fc):
    global LAST_EXEC_NS
    from concourse.bass_utils import run_bass_kernel_spmd

    if float(np.asarray(tf_prob)) >= _RAND_MIN:
        return _kernel_numpy_fallback(
            np.asarray(initial_input), np.asarray(h0), np.asarray(c0),
            np.asarray(targets), float(np.asarray(tf_prob)),
            np.asarray(w_ih), np.asarray(w_hh), np.asarray(b_ih),
            np.asarray(b_hh), np.asarray(w_fc), np.asarray(b_fc))

    mm_dt_name = os.environ.get("BASS_LSTM_DT", "bf16")
    n_cores = int(os.environ.get("BASS_LSTM_CORES", "8"))

    if mm_dt_name not in _prog_cache:
        _prog_cache[mm_dt_name] = _build_program(mm_dt_name)
    nc = _prog_cache[mm_dt_name]

    in_map, out0 = _prepare_host_inputs(
        np.asarray(initial_input), np.asarray(h0), np.asarray(c0),
        np.asarray(w_ih), np.asarray(w_hh), np.asarray(b_ih),
        np.asarray(b_hh), np.asarray(w_fc), np.asarray(b_fc), mm_dt_name,
    )

    # The time recurrence is strictly sequential and, with the fused
    # weight matrix, PE-bound on a single core; cross-core exchange per
    # step (ncfw collectives ~5us floor) costs more than it saves. All 8
    # cores run the same program; core 0's output is used.
    core_ids = list(range(n_cores))
    res = run_bass_kernel_spmd(nc, [in_map] * n_cores, core_ids=core_ids)
    LAST_EXEC_NS = res.exec_time_ns
    outs_dev = res.results[0]["outs"].reshape(T, B, L)[:T - 1]

    out = np.empty((B, 1, T, L), np.float32)
    out[:, 0, 0, :] = out0
    out[:, 0, 1:, :] = np.swapaxes(outs_dev, 0, 1)
    return out
